# revision 47
# baseline (speedup 1.0000x reference)
"""CaptioningRNN forward loss on 8 Trainium2 NeuronCores.

Strategy:
  - The LSTM recurrence is replicated on all 8 cores; the large output
    projection h @ W_vocab (128 x 32 x 512 x 32000) is sharded over the
    vocab axis: each core holds 4000 columns as fp8 * 64 (DoubleRow
    matmuls) and accumulates sum_v exp(logit_v) on-chip via the ACT
    engine's accumulate output, so logits never hit HBM. The target
    logit is computed exactly per row as a DVE dot product with
    host-gathered target columns.
  - All gate nonlinearities are tanh (sigmoid(x) = (tanh(x/2)+1)/2) so
    the ACT engine stays on one activation-table set (exp+tanh): no
    LoadActFuncSet thrash between the gates and the vocab exp. The
    state is carried scaled (c_t = 2c, h = 2h) which makes the gate
    combination the same DVE op count as the sigmoid form; consumers
    descale (W_proj/b_proj doubled, Wh halved, wt halved, exp scale
    halved).
  - One [128, 2048] PSUM tile holds all gate pre-activations in the
    natural [i|f|o|g] layout; the g-block weight columns are doubled on
    host so a single tanh over all 2048 columns at scale s/2 yields
    [tanh(z_ifo/2) | tanh(z_g)] in one ACT instruction.
  - exp work for h_{t-1} overlaps step t's recurrence; the ACT engine
    is the bottleneck and stays ~100% busy.
  - Host combines: loss = sum(mask * (log(sum_cores S) - tgt)) / N.
"""

import numpy as np
import ml_dtypes

import concourse.bass as bass
import concourse.tile as tile
from concourse import mybir, bacc, tile_rust
from concourse.bass_utils import run_bass_kernel_spmd

F32 = mybir.dt.float32
F32R = mybir.dt.float32r
BF16 = mybir.dt.bfloat16
FP8 = mybir.dt.float8e4

# Problem shape (hardcoded per task spec)
N = 128          # batch
T1 = 32          # caption steps (T-1)
D_FEAT = 1280
W_DIM = 256
H = 512
V = 32000
NCORES = 8
VS = V // NCORES          # 4000 vocab cols per core
NSL = 4                   # vocab slices per core (1000 cols each)
SL = VS // NSL            # 1000 cols per slice (2 x 500 halves)
WV_SCALE = 64.0           # W_vocab fp8 scale (descaled in ACT exp)
X_SCALE = 16.0            # x_t fp8 scale   (LSTM fp8 path)
WX_SCALE = 4.0            # Wx fp8 scale    (X_SCALE*WX_SCALE == WV_SCALE)
LSTM_FP8 = True
NULL = 0

_CACHE = {}


def _build(zero_b, zero_bp, zero_bv, repeats=1, lstm_fp8=LSTM_FP8):
    nc = bacc.Bacc("TRN2", target_bir_lowering=False, debug=False)

    if lstm_fp8:
        xt_d = nc.dram_tensor("xt8", [T1, 128, 2, 128], FP8,
                              kind="ExternalInput")
        wb_d = nc.dram_tensor("wb8", [128, 6, 4 * H], FP8,
                              kind="ExternalInput")
    else:
        xt_d = nc.dram_tensor("xt", [T1, 2, 128, 128], F32R,
                              kind="ExternalInput")
        wb_d = nc.dram_tensor("wb", [6, 128, 4 * H], F32R,
                              kind="ExternalInput")
    hx0_d = nc.dram_tensor("hx0", [128, 6, 128], FP8, kind="ExternalInput")
    if not lstm_fp8:
        ht0_d = nc.dram_tensor("ht0", [128, H], F32R, kind="ExternalInput")
    wv_d = nc.dram_tensor("wv8", [128, 4, VS], FP8, kind="ExternalInput")
    wt_d = nc.dram_tensor("wt", [T1, 128, H], F32, kind="ExternalInput")
    id_d = nc.dram_tensor("ident", [128, 128], F32, kind="ExternalInput")
    if not zero_b:
        ones_d = nc.dram_tensor("ones", [1, 128], F32R, kind="ExternalInput")
        bvec_d = nc.dram_tensor("bvec", [1, 4 * H], F32R, kind="ExternalInput")
    if not zero_bv:
        ebv_d = nc.dram_tensor("ebv", [1, VS], F32, kind="ExternalInput")
        bt_d = nc.dram_tensor("bt", [128, T1], F32, kind="ExternalInput")
    s_d = nc.dram_tensor("S_out", [128, T1], F32, kind="ExternalOutput")
    tgt_d = nc.dram_tensor("tgt_out", [128, T1], F32, kind="ExternalOutput")

    GSCALE = (1.0 / WV_SCALE) if lstm_fp8 else 1.0
    AF = mybir.ActivationFunctionType
    DR = mybir.MatmulPerfMode.DoubleRow
    MUL = mybir.AluOpType.mult
    ADD = mybir.AluOpType.add
    with tile.TileContext(nc) as tc:
        with tc.tile_pool(name="const", bufs=1) as constp, \
             tc.tile_pool(name="wbp", bufs=1) as wbp, \
             tc.tile_pool(name="xk", bufs=3) as xkp, \
             tc.tile_pool(name="wtp", bufs=3) as wtp, \
             tc.tile_pool(name="hpool", bufs=3) as hp, \
             tc.tile_pool(name="gates", bufs=2) as gp, \
             tc.tile_pool(name="scr", bufs=4) as scrp, \
             tc.tile_pool(name="psA", bufs=1, space="PSUM") as psA, \
             tc.tile_pool(name="psV", bufs=2, space="PSUM") as psV:

            # --- resident weights; order = DMA serialization order --------
            # (h0 is computed on HOST and shipped as the tiny packed hx0
            # tile first, so the kernel's critical prefix is just wb8)
            hx_first = hp.tile([128, 6, 128], FP8, tag="hx8")
            nc.sync.dma_start(out=hx_first, in_=hx0_d[:, :, :])
            if lstm_fp8:
                wb8 = constp.tile([128, 6, 4 * H], FP8, tag="wb8")
                nc.sync.dma_start(out=wb8, in_=wb_d[:, :, :])
            else:
                wb_sb = []
                for k in range(6):
                    wbt = wbp.tile([128, 4 * H], F32R, tag=f"wb{k}")
                    nc.sync.dma_start(out=wbt, in_=wb_d[k])
                    wb_sb.append(wbt)
            ident = constp.tile([128, 128], F32, tag="ident")
            nc.sync.dma_start(out=ident, in_=id_d[:, :])
            if not zero_b:
                ones_sb = constp.tile([1, 128], F32R, tag="ones")
                nc.sync.dma_start(out=ones_sb, in_=ones_d[:, :])
                bvec_sb = constp.tile([1, 4 * H], F32R, tag="bvec")
                nc.sync.dma_start(out=bvec_sb, in_=bvec_d[:, :])

            # persistent state
            c_t = constp.tile([128, H], F32, tag="c")
            S_acc = constp.tile([128, T1], F32, tag="Sacc")
            tgt_acc = constp.tile([128, T1], F32, tag="tgtacc")

            for _rep in range(repeats):
                nc.vector.memset(c_t, 0.0)

                def emit_vocab(hx8, vs, Ssl):
                    # one 1000-col vocab slice: two fused 4-row (2-pair) DR
                    # matmuls, then one exp with accumulate. h carries 2x ->
                    # descale by 2*WV_SCALE.
                    pV = psV.tile([128, 2, 512], F32, tag="pV")
                    for k in range(2):
                        for hh in range(2):
                            nc.tensor.matmul(
                                pV[:, hh, 0:500],
                                hx8[:, 2 + 2 * k:4 + 2 * k, :],
                                wv8[:, 2 * k:2 * k + 2,
                                    vs * SL + hh * 500:
                                    vs * SL + (hh + 1) * 500],
                                start=(k == 0), stop=(k == 1),
                                perf_mode=DR)
                    ex = scrp.tile([128, 2, 500], F32, tag="ex")
                    if zero_bv:
                        return nc.scalar.activation(
                            ex, pV[:, :, 0:500], AF.Exp,
                            scale=1.0 / (2.0 * WV_SCALE),
                            accum_out=Ssl[:, vs:vs + 1])
                    else:
                        nc.scalar.activation(
                            ex, pV[:, :, 0:500], AF.Exp,
                            scale=1.0 / (2.0 * WV_SCALE))
                        exw = scrp.tile([128, 2, 500], F32, tag="exw")
                        ebv_v = ebv_sb.rearrange("p (s hh m) -> p s hh m",
                                                 s=NSL, hh=2)
                        nc.vector.tensor_mul(exw, ex, ebv_v[:, vs])
                        nc.vector.tensor_reduce(
                            out=Ssl[:, vs:vs + 1], in_=exw,
                            axis=mybir.AxisListType.XY, op=mybir.AluOpType.add)

                def lstm_mms(A, hx8, xk, hT_lhs):
                    # A [128, 2048] = [i|f|o|g] pre-activations (g cols 2x).
                    # fp8: x_t and h^T live packed in one [128, 6, 128] tile
                    # so each 512-col chunk is ONE 6-row (3-pair) DR matmul.
                    nbias = 0 if zero_b else 1
                    if lstm_fp8:
                        for k in range(3):
                            for hh in range(4):
                                nc.tensor.matmul(
                                    A[:, hh * H:(hh + 1) * H],
                                    hx8[:, 2 * k:2 * k + 2, :],
                                    wb8[:, 2 * k:2 * k + 2,
                                        hh * H:(hh + 1) * H],
                                    start=(k == 0),
                                    stop=(k == 2 and nbias == 0),
                                    perf_mode=DR)
                    else:
                        lhs = [xk[0], xk[1],
                               hT_lhs[:, 0:128], hT_lhs[:, 128:256],
                               hT_lhs[:, 256:384], hT_lhs[:, 384:512]]
                        for k in range(6):
                            for hh in range(4):
                                nc.tensor.matmul(
                                    A[:, hh * H:(hh + 1) * H], lhs[k],
                                    wb_sb[k][:, hh * H:(hh + 1) * H],
                                    start=(k == 0),
                                    stop=(k == 5 and nbias == 0))
                    if not zero_b:
                        for hh in range(4):
                            nc.tensor.matmul(
                                A[:, hh * H:(hh + 1) * H], ones_sb,
                                bvec_sb[:, hh * H:(hh + 1) * H],
                                start=False, stop=True)

                # stream-load helpers: x_{t} goes straight into the packed
                # [128, 6, 128] hx tile (rows 0:2) whose rows 2:6 later get
                # h_{t-1}^T; wt is the gathered target-column tile.
                xks, wts = {}, {}

                def fetch_x(t, hx):
                    if lstm_fp8:
                        if t < T1:
                            nc.sync.dma_start(out=hx[:, 0:2, :], in_=xt_d[t])
                    else:
                        if t < T1:
                            xk0 = xkp.tile([128, 128], F32R, tag="xk0")
                            nc.sync.dma_start(out=xk0, in_=xt_d[t, 0])
                            xk1 = xkp.tile([128, 128], F32R, tag="xk1")
                            nc.sync.dma_start(out=xk1, in_=xt_d[t, 1])
                            xks[t] = (xk0, xk1)

                def fetch_wt(t):
                    wt_t = wtp.tile([128, H], F32, tag="wt")
                    nc.sync.dma_start(out=wt_t, in_=wt_d[t])
                    wts[t] = wt_t

                # --- h0 (host-computed, carried as 2*h0): packed hx0 ------
                if _rep == 0:
                    hx_prev = hx_first
                else:
                    hx_prev = hp.tile([128, 6, 128], FP8, tag="hx8")
                    nc.sync.dma_start(out=hx_prev, in_=hx0_d[:, :, :])
                hT_prev = None
                if not lstm_fp8:
                    hT_prev = hp.tile([128, H], F32R, tag="hT")
                    nc.sync.dma_start(out=hT_prev, in_=ht0_d[:, :])
                    fetch_x(0, None)
                fetch_wt(0)
                if _rep == 0:
                    wv8 = constp.tile([128, 4, VS], FP8, tag="wv8")
                    for vs in range(NSL):
                        nc.sync.dma_start(
                            out=wv8[:, :, vs * SL:(vs + 1) * SL],
                            in_=wv_d[:, :, vs * SL:(vs + 1) * SL])
                if not zero_bv and _rep == 0:
                    ebv_sb = constp.tile([128, VS], F32, tag="ebv")
                    nc.sync.dma_start(
                        out=ebv_sb,
                        in_=bass.AP(tensor=ebv_d, offset=0,
                                    ap=[[0, 128], [1, VS]]))
                    bt_sb = constp.tile([128, T1], F32, tag="bt")
                    nc.sync.dma_start(out=bt_sb, in_=bt_d[:, :])
                Ssl_prev = None
                e_last = None

                for t in range(T1):
                    if t + 1 < T1:
                        fetch_wt(t + 1)
                    xk = xks.pop(t, None)
                    wt_t = wts.pop(t)

                    A = psA.tile([128, 2048], F32, tag="A")
                    lstm_mms(A, hx_prev, xk, hT_prev)

                    # gates in [f|i|g|o] layout, tanh'd in two halves so the
                    # v-stt (needs f) overlaps the second tanh; exps of
                    # h_{t-1} are threaded into the ACT queue so it stays
                    # busy through the DVE/PE recurrence-latency windows
                    th = gp.tile([128, 2048], F32, tag="th")
                    i_tanh1 = nc.scalar.activation(th[:, 0:1024],
                                                   A[:, 0:1024],
                                                   AF.Tanh,
                                                   scale=GSCALE * 0.5)
                    if e_last is not None:
                        # resync: all of step t-1's exps drain before this
                        # step's gate tanh, so a one-off slip (e.g. a late
                        # wv8 chunk) cannot permanently shift exps into the
                        # next period where they'd preempt the gate chain
                        tile_rust.add_dep_helper(
                            i_tanh1.ins, e_last.ins,
                            reason="gate tanh after prev exps")
                    i_tanh2 = nc.scalar.activation(th[:, 1024:2048],
                                                   A[:, 1024:2048],
                                                   AF.Tanh,
                                                   scale=GSCALE * 0.5)
                    v = gp.tile([128, H], F32, tag="v")
                    nc.vector.scalar_tensor_tensor(
                        v, th[:, 0:512], 1.0, c_t, op0=ADD, op1=MUL)
                    if t >= 1:
                        e = emit_vocab(hx_prev, 0, Ssl_prev)
                        if e is not None:
                            tile_rust.add_dep_helper(
                                e.ins, i_tanh2.ins,
                                reason="exp0 after gate tanh2")
                    u = gp.tile([128, H], F32, tag="u")
                    nc.vector.scalar_tensor_tensor(
                        u, th[:, 512:1024], 1.0, th[:, 1024:1536],
                        op0=ADD, op1=MUL)
                    nc.vector.scalar_tensor_tensor(
                        c_t, v, 0.5, u, op0=MUL, op1=ADD)
                    tc_ = gp.tile([128, H], F32, tag="tc")
                    i_tanhC = nc.scalar.activation(tc_, c_t, AF.Tanh,
                                                   scale=0.5)
                    if t >= 1:
                        # explicit edges stop the greedy scheduler from
                        # hoisting these exps ahead of tanhC on the in-order
                        # ACT queue (which would delay the recurrence chain)
                        for vs in (1, 2, 3):
                            e = emit_vocab(hx_prev, vs, Ssl_prev)
                            if e is not None:
                                tile_rust.add_dep_helper(
                                    e.ins, i_tanhC.ins,
                                    reason="exp after tanhC")
                                e_last = e
                        nc.vector.tensor_reduce(
                            out=S_acc[:, t - 1:t], in_=Ssl_prev,
                            axis=mybir.AxisListType.X, op=mybir.AluOpType.add)
                    h_new = hp.tile([128, H], F32, tag="h")
                    nc.vector.scalar_tensor_tensor(
                        h_new, th[:, 1536:2048], 1.0, tc_, op0=ADD, op1=MUL)

                    hx_new = hp.tile([128, 6, 128], FP8, tag="hx8")
                    fetch_x(t + 1, hx_new)
                    hT_new = (None if lstm_fp8 else
                              hp.tile([128, H], F32R, tag="hT"))
                    for b in range(4):
                        nc.tensor.transpose(
                            A[:, 1536 + b * 128:1536 + (b + 1) * 128],
                            h_new[:, b * 128:(b + 1) * 128], ident)
                    i_copy = nc.vector.tensor_copy(hx_new[:, 2:6, :],
                                                   A[:, 1536:2048])
                    if hT_new is not None:
                        nc.vector.tensor_copy(hT_new, A[:, 1536:2048])

                    # target logit: tgt[n] = h_new[n, :] . wt_t[n, :] (+ bt)
                    # dep edges keep these off the h->hx8 recurrence chain
                    prod = scrp.tile([128, H], F32, tag="prod")
                    i_prod = nc.vector.tensor_mul(prod, h_new, wt_t)
                    tile_rust.add_dep_helper(i_prod.ins, i_copy.ins,
                                             reason="tgt after hx copy")
                    if zero_bv:
                        nc.vector.tensor_reduce(
                            out=tgt_acc[:, t:t + 1], in_=prod,
                            axis=mybir.AxisListType.X, op=mybir.AluOpType.add)
                    else:
                        tred = scrp.tile([128, 1], F32, tag="tred")
                        nc.vector.tensor_reduce(
                            out=tred, in_=prod,
                            axis=mybir.AxisListType.X, op=mybir.AluOpType.add)
                        nc.vector.tensor_add(
                            tgt_acc[:, t:t + 1], tred, bt_sb[:, t:t + 1])

                    Ssl_prev = scrp.tile([128, NSL], F32, tag="Ssl")
                    hx_prev, hT_prev = hx_new, hT_new

                for vs in range(NSL):
                    emit_vocab(hx_prev, vs, Ssl_prev)
                nc.vector.tensor_reduce(
                    out=S_acc[:, T1 - 1:T1], in_=Ssl_prev,
                    axis=mybir.AxisListType.X, op=mybir.AluOpType.add)

            nc.sync.dma_start(out=s_d[:, :], in_=S_acc)
            nc.sync.dma_start(out=tgt_d[:, :], in_=tgt_acc)

    nc.finalize()
    return nc


def _prep_inputs(features, captions, W_proj, b_proj, W_embed, Wx, Wh, b,
                 W_vocab, b_vocab, lstm_fp8=LSTM_FP8):
    features = np.asarray(features, dtype=np.float32)
    captions = np.asarray(captions)
    W_proj = np.asarray(W_proj, dtype=np.float32)
    b_proj = np.asarray(b_proj, dtype=np.float32)
    W_embed = np.asarray(W_embed, dtype=np.float32)
    Wx = np.asarray(Wx, dtype=np.float32)
    Wh = np.asarray(Wh, dtype=np.float32)
    b = np.asarray(b, dtype=np.float32)
    W_vocab = np.asarray(W_vocab, dtype=np.float32)
    b_vocab = np.asarray(b_vocab, dtype=np.float32)

    captions_in = captions[:, :-1].astype(np.int64)
    captions_out = captions[:, 1:].astype(np.int64)

    zero_b = bool(np.all(b == 0))
    zero_bp = bool(np.all(b_proj == 0))
    zero_bv = bool(np.all(b_vocab == 0))

    x_emb = W_embed[captions_in]                            # [128, 32, 256]
    # device h/c state carries a 2x factor (all-tanh gates): h0 doubled,
    # Wh halved, and consumers of h descale by 0.5. h0 itself is computed
    # here on host and shipped as the packed (x_0 | 2*h0^T) fp8 tile.
    h0 = (features @ W_proj + b_proj) * 2.0                 # [128, 512]
    h0T = np.ascontiguousarray(h0.T)                        # [512, 128]
    # gathered target columns: wt[t, n, :] = 0.5 * W_vocab[:, captions_out]
    wt = np.ascontiguousarray(
        W_vocab.T[captions_out].transpose(1, 0, 2)) * 0.5   # [32, 128, 512]
    ident = np.eye(128, dtype=np.float32)

    # gate cols permuted to [f|i|g|o]; g block doubled so tanh at scale s/2
    # gives tanh(z/2) on f,i,o and tanh(z_g) on g.
    perm = np.concatenate([
        np.arange(H, 2 * H), np.arange(0, H),
        np.arange(3 * H, 4 * H), np.arange(2 * H, 3 * H)])
    gmul = np.ones(4 * H, dtype=np.float32)
    gmul[2 * H:3 * H] = 2.0

    # packed hx0: rows 0:2 = x_0 (fp8-scaled), rows 2:6 = (2*h0)^T
    hx0 = np.zeros((128, 6, 128), dtype=np.float32)
    hx0[:, 2:6, :] = h0T.reshape(4, 128, 128).transpose(1, 0, 2)
    common = {"wt": wt, "ident": ident}
    if lstm_fp8:
        Wb = np.concatenate(
            [Wx * WX_SCALE, Wh * (WV_SCALE * 0.5)], axis=0)[:, perm] * gmul
        common["wb8"] = np.ascontiguousarray(
            Wb.reshape(6, 128, 4 * H).transpose(1, 0, 2)
        ).astype(ml_dtypes.float8_e4m3)
        xt8 = np.ascontiguousarray(
            (x_emb * X_SCALE).transpose(1, 2, 0)
            .reshape(T1, 2, 128, 128).transpose(0, 2, 1, 3)
        ).astype(ml_dtypes.float8_e4m3)
        common["xt8"] = xt8
        hx0[:, 0:2, :] = xt8[0].astype(np.float32)
    else:
        Wb = np.concatenate([Wx, Wh * 0.5], axis=0)[:, perm] * gmul
        common["wb"] = np.ascontiguousarray(Wb.reshape(6, 128, 4 * H))
        common["xt"] = np.ascontiguousarray(
            x_emb.transpose(1, 2, 0).reshape(T1, 2, 128, 128))
        common["ht0"] = np.ascontiguousarray(
            h0T.reshape(4, 128, 128).transpose(1, 0, 2).reshape(128, H))
    common["hx0"] = hx0.astype(ml_dtypes.float8_e4m3)
    if not zero_b:
        common["ones"] = np.ones((1, 128), dtype=np.float32)
        bscale = WV_SCALE if lstm_fp8 else 1.0
        common["bvec"] = (b[perm] * bscale * gmul).reshape(1, 4 * H)
    if not zero_bv:
        common["bt"] = np.ascontiguousarray(
            b_vocab[captions_out].astype(np.float32))       # [128, 32]

    in_maps = []
    for c in range(NCORES):
        m = dict(common)
        wv_shard = (W_vocab[:, c * VS:(c + 1) * VS] * WV_SCALE)
        m["wv8"] = np.ascontiguousarray(
            wv_shard.reshape(4, 128, VS).transpose(1, 0, 2)
        ).astype(ml_dtypes.float8_e4m3)
        if not zero_bv:
            m["ebv"] = np.exp(
                b_vocab[c * VS:(c + 1) * VS]).reshape(1, VS).astype(np.float32)
        in_maps.append(m)
    return in_maps, captions_out, (zero_b, zero_bp, zero_bv)


def kernel(features, captions, W_proj, b_proj, W_embed, Wx, Wh, b,
           W_vocab, b_vocab):
    in_maps, captions_out, key = _prep_inputs(
        features, captions, W_proj, b_proj, W_embed, Wx, Wh, b,
        W_vocab, b_vocab)
    if key not in _CACHE:
        _CACHE[key] = _build(*key)
    nc = _CACHE[key]

    res = run_bass_kernel_spmd(nc, in_maps, core_ids=list(range(NCORES)))
    global last_results
    last_results = res

    S_total = np.zeros((128, T1), dtype=np.float64)
    for c in range(NCORES):
        S_total += res.results[c]["S_out"].astype(np.float64)
    tgt = res.results[0]["tgt_out"].astype(np.float64)      # [128, 32]
    lse = np.log(S_total)
    mask = (captions_out != NULL)
    loss = (np.where(mask, lse - tgt, 0.0)).sum() / N
    return np.float32(loss)


# revision 48
# speedup vs baseline: 22.4158x; 22.4158x over previous
"""CaptioningRNN forward loss on 8 Trainium2 NeuronCores.

Strategy:
  - The LSTM recurrence is replicated on all 8 cores; the large output
    projection h @ W_vocab (128 x 32 x 512 x 32000) is sharded over the
    vocab axis: each core holds 4000 columns as fp8 * 64 (DoubleRow
    matmuls) and accumulates sum_v exp(logit_v) on-chip via the ACT
    engine's accumulate output, so logits never hit HBM. The target
    logit is computed exactly per row as a DVE dot product with
    host-gathered target columns.
  - All gate nonlinearities are tanh (sigmoid(x) = (tanh(x/2)+1)/2) so
    the ACT engine stays on one activation-table set (exp+tanh): no
    LoadActFuncSet thrash between the gates and the vocab exp. The
    state is carried scaled (c_t = 2c, h = 2h) which makes the gate
    combination the same DVE op count as the sigmoid form; consumers
    descale (W_proj/b_proj doubled, Wh halved, wt halved, exp scale
    halved).
  - One [128, 2048] PSUM tile holds all gate pre-activations in the
    natural [i|f|o|g] layout; the g-block weight columns are doubled on
    host so a single tanh over all 2048 columns at scale s/2 yields
    [tanh(z_ifo/2) | tanh(z_g)] in one ACT instruction.
  - exp work for h_{t-1} overlaps step t's recurrence; the ACT engine
    is the bottleneck and stays ~100% busy.
  - Host combines: loss = sum(mask * (log(sum_cores S) - tgt)) / N.
"""

import numpy as np
import ml_dtypes

import concourse.bass as bass
import concourse.tile as tile
from concourse import mybir, bacc, tile_rust
from concourse.bass_utils import run_bass_kernel_spmd

F32 = mybir.dt.float32
F32R = mybir.dt.float32r
BF16 = mybir.dt.bfloat16
FP8 = mybir.dt.float8e4

# Problem shape (hardcoded per task spec)
N = 128          # batch
T1 = 32          # caption steps (T-1)
D_FEAT = 1280
W_DIM = 256
H = 512
V = 32000
NCORES = 8
VS = V // NCORES          # 4000 vocab cols per core
NSL = 4                   # vocab slices per core (1000 cols each)
SL = VS // NSL            # 1000 cols per slice (2 x 500 halves)
WV_SCALE = 64.0           # W_vocab fp8 scale (descaled in ACT exp)
X_SCALE = 16.0            # x_t fp8 scale   (LSTM fp8 path)
WX_SCALE = 4.0            # Wx fp8 scale    (X_SCALE*WX_SCALE == WV_SCALE)
LSTM_FP8 = True
NULL = 0

_CACHE = {}


def _build(zero_b, zero_bp, zero_bv, repeats=1, lstm_fp8=LSTM_FP8):
    nc = bacc.Bacc("TRN2", target_bir_lowering=False, debug=False)

    if lstm_fp8:
        xt_d = nc.dram_tensor("xt8", [T1, 128, 2, 128], FP8,
                              kind="ExternalInput")
        wb_d = nc.dram_tensor("wb8", [128, 6, 4 * H], FP8,
                              kind="ExternalInput")
    else:
        xt_d = nc.dram_tensor("xt", [T1, 2, 128, 128], F32R,
                              kind="ExternalInput")
        wb_d = nc.dram_tensor("wb", [6, 128, 4 * H], F32R,
                              kind="ExternalInput")
    hx0_d = nc.dram_tensor("hx0", [128, 6, 128], FP8, kind="ExternalInput")
    if not lstm_fp8:
        ht0_d = nc.dram_tensor("ht0", [128, H], F32R, kind="ExternalInput")
    wv_d = nc.dram_tensor("wv8", [128, 4, VS], FP8, kind="ExternalInput")
    wt_d = nc.dram_tensor("wt", [T1, 128, H], F32, kind="ExternalInput")
    id_d = nc.dram_tensor("ident", [128, 128], F32, kind="ExternalInput")
    if not zero_b:
        ones_d = nc.dram_tensor("ones", [1, 128], F32R, kind="ExternalInput")
        bvec_d = nc.dram_tensor("bvec", [1, 4 * H], F32R, kind="ExternalInput")
    if not zero_bv:
        ebv_d = nc.dram_tensor("ebv", [1, VS], F32, kind="ExternalInput")
        bt_d = nc.dram_tensor("bt", [128, T1], F32, kind="ExternalInput")
    s_d = nc.dram_tensor("S_out", [128, T1], F32, kind="ExternalOutput")
    tgt_d = nc.dram_tensor("tgt_out", [128, T1], F32, kind="ExternalOutput")

    GSCALE = (1.0 / WV_SCALE) if lstm_fp8 else 1.0
    AF = mybir.ActivationFunctionType
    DR = mybir.MatmulPerfMode.DoubleRow
    MUL = mybir.AluOpType.mult
    ADD = mybir.AluOpType.add
    with tile.TileContext(nc) as tc:
        with tc.tile_pool(name="const", bufs=1) as constp, \
             tc.tile_pool(name="wbp", bufs=1) as wbp, \
             tc.tile_pool(name="xk", bufs=3) as xkp, \
             tc.tile_pool(name="wtp", bufs=3) as wtp, \
             tc.tile_pool(name="hpool", bufs=3) as hp, \
             tc.tile_pool(name="gates", bufs=2) as gp, \
             tc.tile_pool(name="scr", bufs=4) as scrp, \
             tc.tile_pool(name="psA", bufs=1, space="PSUM") as psA, \
             tc.tile_pool(name="psV", bufs=2, space="PSUM") as psV:

            # --- resident weights; order = DMA serialization order --------
            # (h0 is computed on HOST and shipped as the tiny packed hx0
            # tile first, so the kernel's critical prefix is just wb8)
            hx_first = hp.tile([128, 6, 128], FP8, tag="hx8")
            nc.sync.dma_start(out=hx_first, in_=hx0_d[:, :, :])
            if lstm_fp8:
                wb8 = constp.tile([128, 6, 4 * H], FP8, tag="wb8")
                nc.sync.dma_start(out=wb8, in_=wb_d[:, :, :])
            else:
                wb_sb = []
                for k in range(6):
                    wbt = wbp.tile([128, 4 * H], F32R, tag=f"wb{k}")
                    nc.sync.dma_start(out=wbt, in_=wb_d[k])
                    wb_sb.append(wbt)
            ident = constp.tile([128, 128], F32, tag="ident")
            nc.sync.dma_start(out=ident, in_=id_d[:, :])
            if not zero_b:
                ones_sb = constp.tile([1, 128], F32R, tag="ones")
                nc.sync.dma_start(out=ones_sb, in_=ones_d[:, :])
                bvec_sb = constp.tile([1, 4 * H], F32R, tag="bvec")
                nc.sync.dma_start(out=bvec_sb, in_=bvec_d[:, :])

            # persistent state
            c_t = constp.tile([128, H], F32, tag="c")
            S_acc = constp.tile([128, T1], F32, tag="Sacc")
            tgt_acc = constp.tile([128, T1], F32, tag="tgtacc")

            for _rep in range(repeats):
                nc.vector.memset(c_t, 0.0)

                def emit_vocab(hx8, vs, Ssl):
                    # one 1000-col vocab slice: two fused 4-row (2-pair) DR
                    # matmuls, then one exp with accumulate. h carries 2x ->
                    # descale by 2*WV_SCALE.
                    pV = psV.tile([128, 2, 512], F32, tag="pV")
                    for k in range(2):
                        for hh in range(2):
                            nc.tensor.matmul(
                                pV[:, hh, 0:500],
                                hx8[:, 2 + 2 * k:4 + 2 * k, :],
                                wv8[:, 2 * k:2 * k + 2,
                                    vs * SL + hh * 500:
                                    vs * SL + (hh + 1) * 500],
                                start=(k == 0), stop=(k == 1),
                                perf_mode=DR)
                    if zero_bv:
                        # exp written in place over the PSUM logits: cheaper
                        # access init than an SBUF output tile
                        return nc.scalar.activation(
                            pV[:, :, 0:500], pV[:, :, 0:500], AF.Exp,
                            scale=1.0 / (2.0 * WV_SCALE),
                            accum_out=Ssl[:, vs:vs + 1])
                    else:
                        ex = scrp.tile([128, 2, 500], F32, tag="ex")
                        nc.scalar.activation(
                            ex, pV[:, :, 0:500], AF.Exp,
                            scale=1.0 / (2.0 * WV_SCALE))
                        exw = scrp.tile([128, 2, 500], F32, tag="exw")
                        ebv_v = ebv_sb.rearrange("p (s hh m) -> p s hh m",
                                                 s=NSL, hh=2)
                        nc.vector.tensor_mul(exw, ex, ebv_v[:, vs])
                        nc.vector.tensor_reduce(
                            out=Ssl[:, vs:vs + 1], in_=exw,
                            axis=mybir.AxisListType.XY, op=mybir.AluOpType.add)

                def lstm_mms(A, hx8, xk, hT_lhs):
                    # A [128, 2048] = [i|f|o|g] pre-activations (g cols 2x).
                    # fp8: x_t and h^T live packed in one [128, 6, 128] tile
                    # so each 512-col chunk is ONE 6-row (3-pair) DR matmul.
                    nbias = 0 if zero_b else 1
                    if lstm_fp8:
                        for k in range(3):
                            for hh in range(4):
                                nc.tensor.matmul(
                                    A[:, hh * H:(hh + 1) * H],
                                    hx8[:, 2 * k:2 * k + 2, :],
                                    wb8[:, 2 * k:2 * k + 2,
                                        hh * H:(hh + 1) * H],
                                    start=(k == 0),
                                    stop=(k == 2 and nbias == 0),
                                    perf_mode=DR)
                    else:
                        lhs = [xk[0], xk[1],
                               hT_lhs[:, 0:128], hT_lhs[:, 128:256],
                               hT_lhs[:, 256:384], hT_lhs[:, 384:512]]
                        for k in range(6):
                            for hh in range(4):
                                nc.tensor.matmul(
                                    A[:, hh * H:(hh + 1) * H], lhs[k],
                                    wb_sb[k][:, hh * H:(hh + 1) * H],
                                    start=(k == 0),
                                    stop=(k == 5 and nbias == 0))
                    if not zero_b:
                        for hh in range(4):
                            nc.tensor.matmul(
                                A[:, hh * H:(hh + 1) * H], ones_sb,
                                bvec_sb[:, hh * H:(hh + 1) * H],
                                start=False, stop=True)

                # stream-load helpers: x_{t} goes straight into the packed
                # [128, 6, 128] hx tile (rows 0:2) whose rows 2:6 later get
                # h_{t-1}^T; wt is the gathered target-column tile.
                xks, wts = {}, {}

                def fetch_x(t, hx):
                    if lstm_fp8:
                        if t < T1:
                            nc.sync.dma_start(out=hx[:, 0:2, :], in_=xt_d[t])
                    else:
                        if t < T1:
                            xk0 = xkp.tile([128, 128], F32R, tag="xk0")
                            nc.sync.dma_start(out=xk0, in_=xt_d[t, 0])
                            xk1 = xkp.tile([128, 128], F32R, tag="xk1")
                            nc.sync.dma_start(out=xk1, in_=xt_d[t, 1])
                            xks[t] = (xk0, xk1)

                def fetch_wt(t):
                    wt_t = wtp.tile([128, H], F32, tag="wt")
                    nc.sync.dma_start(out=wt_t, in_=wt_d[t])
                    wts[t] = wt_t

                # --- h0 (host-computed, carried as 2*h0): packed hx0 ------
                if _rep == 0:
                    hx_prev = hx_first
                else:
                    hx_prev = hp.tile([128, 6, 128], FP8, tag="hx8")
                    nc.sync.dma_start(out=hx_prev, in_=hx0_d[:, :, :])
                hT_prev = None
                if not lstm_fp8:
                    hT_prev = hp.tile([128, H], F32R, tag="hT")
                    nc.sync.dma_start(out=hT_prev, in_=ht0_d[:, :])
                    fetch_x(0, None)
                fetch_wt(0)
                if _rep == 0:
                    wv8 = constp.tile([128, 4, VS], FP8, tag="wv8")
                    for vs in range(NSL):
                        nc.sync.dma_start(
                            out=wv8[:, :, vs * SL:(vs + 1) * SL],
                            in_=wv_d[:, :, vs * SL:(vs + 1) * SL])
                if not zero_bv and _rep == 0:
                    ebv_sb = constp.tile([128, VS], F32, tag="ebv")
                    nc.sync.dma_start(
                        out=ebv_sb,
                        in_=bass.AP(tensor=ebv_d, offset=0,
                                    ap=[[0, 128], [1, VS]]))
                    bt_sb = constp.tile([128, T1], F32, tag="bt")
                    nc.sync.dma_start(out=bt_sb, in_=bt_d[:, :])
                Ssl_prev = None
                e_last = None

                for t in range(T1):
                    if t + 1 < T1:
                        fetch_wt(t + 1)
                    xk = xks.pop(t, None)
                    wt_t = wts.pop(t)

                    A = psA.tile([128, 2048], F32, tag="A")
                    lstm_mms(A, hx_prev, xk, hT_prev)

                    # gates in [f|i|g|o] layout, tanh'd in two halves so the
                    # v-stt (needs f) overlaps the second tanh; exps of
                    # h_{t-1} are threaded into the ACT queue so it stays
                    # busy through the DVE/PE recurrence-latency windows
                    th = gp.tile([128, 2048], F32, tag="th")
                    i_tanh1 = nc.scalar.activation(th[:, 0:1024],
                                                   A[:, 0:1024],
                                                   AF.Tanh,
                                                   scale=GSCALE * 0.5)
                    if e_last is not None:
                        # resync: all of step t-1's exps drain before this
                        # step's gate tanh, so a one-off slip (e.g. a late
                        # wv8 chunk) cannot permanently shift exps into the
                        # next period where they'd preempt the gate chain
                        tile_rust.add_dep_helper(
                            i_tanh1.ins, e_last.ins,
                            reason="gate tanh after prev exps")
                    i_tanh2 = nc.scalar.activation(th[:, 1024:2048],
                                                   A[:, 1024:2048],
                                                   AF.Tanh,
                                                   scale=GSCALE * 0.5)
                    v = gp.tile([128, H], F32, tag="v")
                    nc.vector.scalar_tensor_tensor(
                        v, th[:, 0:512], 1.0, c_t, op0=ADD, op1=MUL)
                    if t >= 1:
                        e = emit_vocab(hx_prev, 0, Ssl_prev)
                        if e is not None:
                            tile_rust.add_dep_helper(
                                e.ins, i_tanh2.ins,
                                reason="exp0 after gate tanh2")
                    u = gp.tile([128, H], F32, tag="u")
                    nc.vector.scalar_tensor_tensor(
                        u, th[:, 512:1024], 1.0, th[:, 1024:1536],
                        op0=ADD, op1=MUL)
                    nc.vector.scalar_tensor_tensor(
                        c_t, v, 0.5, u, op0=MUL, op1=ADD)
                    tc_ = gp.tile([128, H], F32, tag="tc")
                    i_tanhC = nc.scalar.activation(tc_, c_t, AF.Tanh,
                                                   scale=0.5)
                    if t >= 1:
                        # explicit edges stop the greedy scheduler from
                        # hoisting these exps ahead of tanhC on the in-order
                        # ACT queue (which would delay the recurrence chain)
                        for vs in (1, 2, 3):
                            e = emit_vocab(hx_prev, vs, Ssl_prev)
                            if e is not None:
                                tile_rust.add_dep_helper(
                                    e.ins, i_tanhC.ins,
                                    reason="exp after tanhC")
                                e_last = e
                        nc.vector.tensor_reduce(
                            out=S_acc[:, t - 1:t], in_=Ssl_prev,
                            axis=mybir.AxisListType.X, op=mybir.AluOpType.add)
                    h_new = hp.tile([128, H], F32, tag="h")
                    nc.vector.scalar_tensor_tensor(
                        h_new, th[:, 1536:2048], 1.0, tc_, op0=ADD, op1=MUL)

                    hx_new = hp.tile([128, 6, 128], FP8, tag="hx8")
                    fetch_x(t + 1, hx_new)
                    hT_new = (None if lstm_fp8 else
                              hp.tile([128, H], F32R, tag="hT"))
                    for b in range(4):
                        nc.tensor.transpose(
                            A[:, 1536 + b * 128:1536 + (b + 1) * 128],
                            h_new[:, b * 128:(b + 1) * 128], ident)
                    i_copy = nc.vector.tensor_copy(hx_new[:, 2:6, :],
                                                   A[:, 1536:2048])
                    if hT_new is not None:
                        nc.vector.tensor_copy(hT_new, A[:, 1536:2048])

                    # target logit: tgt[n] = h_new[n, :] . wt_t[n, :] (+ bt)
                    # dep edges keep these off the h->hx8 recurrence chain
                    prod = scrp.tile([128, H], F32, tag="prod")
                    i_prod = nc.vector.tensor_mul(prod, h_new, wt_t)
                    tile_rust.add_dep_helper(i_prod.ins, i_copy.ins,
                                             reason="tgt after hx copy")
                    if zero_bv:
                        nc.vector.tensor_reduce(
                            out=tgt_acc[:, t:t + 1], in_=prod,
                            axis=mybir.AxisListType.X, op=mybir.AluOpType.add)
                    else:
                        tred = scrp.tile([128, 1], F32, tag="tred")
                        nc.vector.tensor_reduce(
                            out=tred, in_=prod,
                            axis=mybir.AxisListType.X, op=mybir.AluOpType.add)
                        nc.vector.tensor_add(
                            tgt_acc[:, t:t + 1], tred, bt_sb[:, t:t + 1])

                    Ssl_prev = scrp.tile([128, NSL], F32, tag="Ssl")
                    hx_prev, hT_prev = hx_new, hT_new

                for vs in range(NSL):
                    emit_vocab(hx_prev, vs, Ssl_prev)
                nc.vector.tensor_reduce(
                    out=S_acc[:, T1 - 1:T1], in_=Ssl_prev,
                    axis=mybir.AxisListType.X, op=mybir.AluOpType.add)

            nc.sync.dma_start(out=s_d[:, :], in_=S_acc)
            nc.sync.dma_start(out=tgt_d[:, :], in_=tgt_acc)

    nc.finalize()
    return nc


def _prep_inputs(features, captions, W_proj, b_proj, W_embed, Wx, Wh, b,
                 W_vocab, b_vocab, lstm_fp8=LSTM_FP8):
    features = np.asarray(features, dtype=np.float32)
    captions = np.asarray(captions)
    W_proj = np.asarray(W_proj, dtype=np.float32)
    b_proj = np.asarray(b_proj, dtype=np.float32)
    W_embed = np.asarray(W_embed, dtype=np.float32)
    Wx = np.asarray(Wx, dtype=np.float32)
    Wh = np.asarray(Wh, dtype=np.float32)
    b = np.asarray(b, dtype=np.float32)
    W_vocab = np.asarray(W_vocab, dtype=np.float32)
    b_vocab = np.asarray(b_vocab, dtype=np.float32)

    captions_in = captions[:, :-1].astype(np.int64)
    captions_out = captions[:, 1:].astype(np.int64)

    zero_b = bool(np.all(b == 0))
    zero_bp = bool(np.all(b_proj == 0))
    zero_bv = bool(np.all(b_vocab == 0))

    x_emb = W_embed[captions_in]                            # [128, 32, 256]
    # device h/c state carries a 2x factor (all-tanh gates): h0 doubled,
    # Wh halved, and consumers of h descale by 0.5. h0 itself is computed
    # here on host and shipped as the packed (x_0 | 2*h0^T) fp8 tile.
    h0 = (features @ W_proj + b_proj) * 2.0                 # [128, 512]
    h0T = np.ascontiguousarray(h0.T)                        # [512, 128]
    # gathered target columns: wt[t, n, :] = 0.5 * W_vocab[:, captions_out]
    wt = np.ascontiguousarray(
        W_vocab.T[captions_out].transpose(1, 0, 2)) * 0.5   # [32, 128, 512]
    ident = np.eye(128, dtype=np.float32)

    # gate cols permuted to [f|i|g|o]; g block doubled so tanh at scale s/2
    # gives tanh(z/2) on f,i,o and tanh(z_g) on g.
    perm = np.concatenate([
        np.arange(H, 2 * H), np.arange(0, H),
        np.arange(3 * H, 4 * H), np.arange(2 * H, 3 * H)])
    gmul = np.ones(4 * H, dtype=np.float32)
    gmul[2 * H:3 * H] = 2.0

    # packed hx0: rows 0:2 = x_0 (fp8-scaled), rows 2:6 = (2*h0)^T
    hx0 = np.zeros((128, 6, 128), dtype=np.float32)
    hx0[:, 2:6, :] = h0T.reshape(4, 128, 128).transpose(1, 0, 2)
    common = {"wt": wt, "ident": ident}
    if lstm_fp8:
        Wb = np.concatenate(
            [Wx * WX_SCALE, Wh * (WV_SCALE * 0.5)], axis=0)[:, perm] * gmul
        common["wb8"] = np.ascontiguousarray(
            Wb.reshape(6, 128, 4 * H).transpose(1, 0, 2)
        ).astype(ml_dtypes.float8_e4m3)
        xt8 = np.ascontiguousarray(
            (x_emb * X_SCALE).transpose(1, 2, 0)
            .reshape(T1, 2, 128, 128).transpose(0, 2, 1, 3)
        ).astype(ml_dtypes.float8_e4m3)
        common["xt8"] = xt8
        hx0[:, 0:2, :] = xt8[0].astype(np.float32)
    else:
        Wb = np.concatenate([Wx, Wh * 0.5], axis=0)[:, perm] * gmul
        common["wb"] = np.ascontiguousarray(Wb.reshape(6, 128, 4 * H))
        common["xt"] = np.ascontiguousarray(
            x_emb.transpose(1, 2, 0).reshape(T1, 2, 128, 128))
        common["ht0"] = np.ascontiguousarray(
            h0T.reshape(4, 128, 128).transpose(1, 0, 2).reshape(128, H))
    common["hx0"] = hx0.astype(ml_dtypes.float8_e4m3)
    if not zero_b:
        common["ones"] = np.ones((1, 128), dtype=np.float32)
        bscale = WV_SCALE if lstm_fp8 else 1.0
        common["bvec"] = (b[perm] * bscale * gmul).reshape(1, 4 * H)
    if not zero_bv:
        common["bt"] = np.ascontiguousarray(
            b_vocab[captions_out].astype(np.float32))       # [128, 32]

    in_maps = []
    for c in range(NCORES):
        m = dict(common)
        wv_shard = (W_vocab[:, c * VS:(c + 1) * VS] * WV_SCALE)
        m["wv8"] = np.ascontiguousarray(
            wv_shard.reshape(4, 128, VS).transpose(1, 0, 2)
        ).astype(ml_dtypes.float8_e4m3)
        if not zero_bv:
            m["ebv"] = np.exp(
                b_vocab[c * VS:(c + 1) * VS]).reshape(1, VS).astype(np.float32)
        in_maps.append(m)
    return in_maps, captions_out, (zero_b, zero_bp, zero_bv)


def kernel(features, captions, W_proj, b_proj, W_embed, Wx, Wh, b,
           W_vocab, b_vocab):
    in_maps, captions_out, key = _prep_inputs(
        features, captions, W_proj, b_proj, W_embed, Wx, Wh, b,
        W_vocab, b_vocab)
    if key not in _CACHE:
        _CACHE[key] = _build(*key)
    nc = _CACHE[key]

    res = run_bass_kernel_spmd(nc, in_maps, core_ids=list(range(NCORES)))
    global last_results
    last_results = res

    S_total = np.zeros((128, T1), dtype=np.float64)
    for c in range(NCORES):
        S_total += res.results[c]["S_out"].astype(np.float64)
    tgt = res.results[0]["tgt_out"].astype(np.float64)      # [128, 32]
    lse = np.log(S_total)
    mask = (captions_out != NULL)
    loss = (np.where(mask, lse - tgt, 0.0)).sum() / N
    return np.float32(loss)


# revision 65
# speedup vs baseline: 22.6843x; 1.0120x over previous
"""CaptioningRNN forward loss on 8 Trainium2 NeuronCores.

Strategy:
  - The LSTM recurrence is replicated on all 8 cores; the large output
    projection h @ W_vocab (128 x 32 x 512 x 32000) is sharded over the
    vocab axis: each core holds 4000 columns as fp8 * 64 (DoubleRow
    matmuls) and accumulates sum_v exp(logit_v) on-chip via the ACT
    engine's accumulate output, so logits never hit HBM. The target
    logit is computed exactly per row as a DVE dot product with
    host-gathered target columns.
  - All gate nonlinearities are tanh (sigmoid(x) = (tanh(x/2)+1)/2) so
    the ACT engine stays on one activation-table set (exp+tanh): no
    LoadActFuncSet thrash between the gates and the vocab exp. The
    state is carried scaled (c_t = 2c, h = 2h) which makes the gate
    combination the same DVE op count as the sigmoid form; consumers
    descale (W_proj/b_proj doubled, Wh halved, wt halved, exp scale
    halved).
  - One [128, 2048] PSUM tile holds all gate pre-activations in the
    natural [i|f|o|g] layout; the g-block weight columns are doubled on
    host so a single tanh over all 2048 columns at scale s/2 yields
    [tanh(z_ifo/2) | tanh(z_g)] in one ACT instruction.
  - exp work for h_{t-1} overlaps step t's recurrence; the ACT engine
    is the bottleneck and stays ~100% busy.
  - Host combines: loss = sum(mask * (log(sum_cores S) - tgt)) / N.
"""

import numpy as np
import ml_dtypes

import concourse.bass as bass
import concourse.tile as tile
from concourse import mybir, bacc, tile_rust
from concourse.bass_utils import run_bass_kernel_spmd

F32 = mybir.dt.float32
F32R = mybir.dt.float32r
BF16 = mybir.dt.bfloat16
FP8 = mybir.dt.float8e4

# Problem shape (hardcoded per task spec)
N = 128          # batch
T1 = 32          # caption steps (T-1)
D_FEAT = 1280
W_DIM = 256
H = 512
V = 32000
NCORES = 8
VS = V // NCORES          # 4000 vocab cols per core
NSL = 4                   # vocab slices per core (1000 cols each)
SL = VS // NSL            # 1000 cols per slice (2 x 500 halves)
WV_SCALE = 64.0           # W_vocab fp8 scale (descaled in ACT exp)
X_SCALE = 16.0            # x_t fp8 scale   (LSTM fp8 path)
WX_SCALE = 4.0            # Wx fp8 scale    (X_SCALE*WX_SCALE == WV_SCALE)
LSTM_FP8 = True
NULL = 0

_CACHE = {}


def _build(zero_b, zero_bp, zero_bv, repeats=1, lstm_fp8=LSTM_FP8):
    nc = bacc.Bacc("TRN2", target_bir_lowering=False, debug=False)

    if lstm_fp8:
        xt_d = nc.dram_tensor("xt8", [T1, 128, 2, 128], FP8,
                              kind="ExternalInput")
        wb_d = nc.dram_tensor("wb8", [128, 6, 4 * H], FP8,
                              kind="ExternalInput")
    else:
        xt_d = nc.dram_tensor("xt", [T1, 2, 128, 128], F32R,
                              kind="ExternalInput")
        wb_d = nc.dram_tensor("wb", [6, 128, 4 * H], F32R,
                              kind="ExternalInput")
    hx0_d = nc.dram_tensor("hx0", [128, 6, 128], FP8, kind="ExternalInput")
    if not lstm_fp8:
        ht0_d = nc.dram_tensor("ht0", [128, H], F32R, kind="ExternalInput")
    wv_d = nc.dram_tensor("wv8", [128, 4, VS], FP8, kind="ExternalInput")
    wt_d = nc.dram_tensor("wt", [T1, 128, H], F32, kind="ExternalInput")
    id_d = nc.dram_tensor("ident", [128, 128], F32, kind="ExternalInput")
    if not zero_b:
        ones_d = nc.dram_tensor("ones", [1, 128], F32R, kind="ExternalInput")
        bvec_d = nc.dram_tensor("bvec", [1, 4 * H], F32R, kind="ExternalInput")
    if not zero_bv:
        ebv_d = nc.dram_tensor("ebv", [1, VS], F32, kind="ExternalInput")
        bt_d = nc.dram_tensor("bt", [128, T1], F32, kind="ExternalInput")
    s_d = nc.dram_tensor("S_out", [128, T1], F32, kind="ExternalOutput")
    tgt_d = nc.dram_tensor("tgt_out", [128, T1], F32, kind="ExternalOutput")

    GSCALE = (1.0 / WV_SCALE) if lstm_fp8 else 1.0
    AF = mybir.ActivationFunctionType
    DR = mybir.MatmulPerfMode.DoubleRow
    MUL = mybir.AluOpType.mult
    ADD = mybir.AluOpType.add
    with tile.TileContext(nc) as tc:
        with tc.tile_pool(name="const", bufs=1) as constp, \
             tc.tile_pool(name="wbp", bufs=1) as wbp, \
             tc.tile_pool(name="xk", bufs=3) as xkp, \
             tc.tile_pool(name="wtp", bufs=3) as wtp, \
             tc.tile_pool(name="hpool", bufs=3) as hp, \
             tc.tile_pool(name="gates", bufs=2) as gp, \
             tc.tile_pool(name="scr", bufs=4) as scrp, \
             tc.tile_pool(name="psA", bufs=1, space="PSUM") as psA, \
             tc.tile_pool(name="psV", bufs=2, space="PSUM") as psV:

            # --- resident weights; order = DMA serialization order --------
            # (h0 is computed on HOST and shipped as the tiny packed hx0
            # tile first, so the kernel's critical prefix is just wb8)
            hx_first = hp.tile([128, 6, 128], FP8, tag="hx8")
            nc.sync.dma_start(out=hx_first, in_=hx0_d[:, :, :])
            if lstm_fp8:
                # 3 pair-chunks: first A matmuls start after chunk 0 lands
                wb8 = constp.tile([128, 6, 4 * H], FP8, tag="wb8")
                for k in range(3):
                    nc.sync.dma_start(out=wb8[:, 2 * k:2 * k + 2, :],
                                      in_=wb_d[:, 2 * k:2 * k + 2, :])
            else:
                wb_sb = []
                for k in range(6):
                    wbt = wbp.tile([128, 4 * H], F32R, tag=f"wb{k}")
                    nc.sync.dma_start(out=wbt, in_=wb_d[k])
                    wb_sb.append(wbt)
            ident = constp.tile([128, 128], F32, tag="ident")
            nc.sync.dma_start(out=ident, in_=id_d[:, :])
            if not zero_b:
                ones_sb = constp.tile([1, 128], F32R, tag="ones")
                nc.sync.dma_start(out=ones_sb, in_=ones_d[:, :])
                bvec_sb = constp.tile([1, 4 * H], F32R, tag="bvec")
                nc.sync.dma_start(out=bvec_sb, in_=bvec_d[:, :])

            # persistent state
            c_t = constp.tile([128, H], F32, tag="c")
            S_acc = constp.tile([128, T1], F32, tag="Sacc")
            tgt_acc = constp.tile([128, T1], F32, tag="tgtacc")

            for _rep in range(repeats):
                nc.vector.memset(c_t, 0.0)

                def emit_vocab(hx8, vs, Ssl, sbuf_out=False):
                    # one 1000-col vocab slice: fused (2-pair-row) DR
                    # matmuls, then one exp with accumulate. h carries 2x ->
                    # descale by 2*WV_SCALE. In-place PSUM output is cheaper
                    # than an SBUF ex tile; sbuf_out picks the slower
                    # variant where the ACT queue would idle anyway.
                    pV = psV.tile([128, 2, 512], F32, tag="pV")
                    for hh in range(2):
                        for k in range(2):
                            nc.tensor.matmul(
                                pV[:, hh, 0:500],
                                hx8[:, 2 + 2 * k:4 + 2 * k, :],
                                wv8[:, 2 * k:2 * k + 2,
                                    vs * SL + hh * 500:
                                    vs * SL + (hh + 1) * 500],
                                start=(k == 0), stop=(k == 1),
                                perf_mode=DR)
                    if zero_bv:
                        if sbuf_out:
                            ex = scrp.tile([128, 2, 500], F32, tag="ex")
                            return nc.scalar.activation(
                                ex, pV[:, :, 0:500], AF.Exp,
                                scale=1.0 / (2.0 * WV_SCALE),
                                accum_out=Ssl[:, vs:vs + 1])
                        return nc.scalar.activation(
                            pV[:, :, 0:500], pV[:, :, 0:500], AF.Exp,
                            scale=1.0 / (2.0 * WV_SCALE),
                            accum_out=Ssl[:, vs:vs + 1])
                    ex = scrp.tile([128, 2, 500], F32, tag="ex")
                    nc.scalar.activation(
                        ex, pV[:, :, 0:500], AF.Exp,
                        scale=1.0 / (2.0 * WV_SCALE))
                    exw = scrp.tile([128, 2, 500], F32, tag="exw")
                    ebv_v = ebv_sb.rearrange("p (s hh m) -> p s hh m",
                                             s=NSL, hh=2)
                    nc.vector.tensor_mul(exw, ex, ebv_v[:, vs])
                    nc.vector.tensor_reduce(
                        out=Ssl[:, vs:vs + 1], in_=exw,
                        axis=mybir.AxisListType.XY, op=mybir.AluOpType.add)
                    return None

                def lstm_mms(A, hx8, xk, hT_lhs):
                    # A [128, 2048] = [i|f|o|g] pre-activations (g cols 2x).
                    # fp8: x_t and h^T live packed in one [128, 6, 128] tile
                    # so each 512-col chunk is ONE 6-row (3-pair) DR matmul.
                    # hh-outer order: gate chunk 0 completes after only 3
                    # matmuls, so the gate tanh starts ~1 us sooner on the
                    # recurrence chain
                    nbias = 0 if zero_b else 1
                    if lstm_fp8:
                        for hh in range(4):
                            for k in range(3):
                                nc.tensor.matmul(
                                    A[:, hh * H:(hh + 1) * H],
                                    hx8[:, 2 * k:2 * k + 2, :],
                                    wb8[:, 2 * k:2 * k + 2,
                                        hh * H:(hh + 1) * H],
                                    start=(k == 0),
                                    stop=(k == 2 and nbias == 0),
                                    perf_mode=DR)
                            if not zero_b:
                                nc.tensor.matmul(
                                    A[:, hh * H:(hh + 1) * H], ones_sb,
                                    bvec_sb[:, hh * H:(hh + 1) * H],
                                    start=False, stop=True)
                    else:
                        lhs = [xk[0], xk[1],
                               hT_lhs[:, 0:128], hT_lhs[:, 128:256],
                               hT_lhs[:, 256:384], hT_lhs[:, 384:512]]
                        for hh in range(4):
                            for k in range(6):
                                nc.tensor.matmul(
                                    A[:, hh * H:(hh + 1) * H], lhs[k],
                                    wb_sb[k][:, hh * H:(hh + 1) * H],
                                    start=(k == 0),
                                    stop=(k == 5 and nbias == 0))
                            if not zero_b:
                                nc.tensor.matmul(
                                    A[:, hh * H:(hh + 1) * H], ones_sb,
                                    bvec_sb[:, hh * H:(hh + 1) * H],
                                    start=False, stop=True)

                # stream-load helpers: x_{t} goes straight into the packed
                # [128, 6, 128] hx tile (rows 0:2) whose rows 2:6 later get
                # h_{t-1}^T; wt is the gathered target-column tile.
                xks, wts = {}, {}

                def fetch_x(t, hx):
                    if lstm_fp8:
                        if t < T1:
                            nc.sync.dma_start(out=hx[:, 0:2, :], in_=xt_d[t])
                    else:
                        if t < T1:
                            xk0 = xkp.tile([128, 128], F32R, tag="xk0")
                            nc.sync.dma_start(out=xk0, in_=xt_d[t, 0])
                            xk1 = xkp.tile([128, 128], F32R, tag="xk1")
                            nc.sync.dma_start(out=xk1, in_=xt_d[t, 1])
                            xks[t] = (xk0, xk1)

                def fetch_wt(t):
                    wt_t = wtp.tile([128, H], F32, tag="wt")
                    nc.sync.dma_start(out=wt_t, in_=wt_d[t])
                    wts[t] = wt_t

                # --- h0 (host-computed, carried as 2*h0): packed hx0 ------
                if _rep == 0:
                    hx_prev = hx_first
                else:
                    hx_prev = hp.tile([128, 6, 128], FP8, tag="hx8")
                    nc.sync.dma_start(out=hx_prev, in_=hx0_d[:, :, :])
                hT_prev = None
                if not lstm_fp8:
                    hT_prev = hp.tile([128, H], F32R, tag="hT")
                    nc.sync.dma_start(out=hT_prev, in_=ht0_d[:, :])
                    fetch_x(0, None)
                fetch_wt(0)
                if _rep == 0:
                    wv8 = constp.tile([128, 4, VS], FP8, tag="wv8")
                    for vs in range(NSL):
                        nc.sync.dma_start(
                            out=wv8[:, :, vs * SL:(vs + 1) * SL],
                            in_=wv_d[:, :, vs * SL:(vs + 1) * SL])
                if not zero_bv and _rep == 0:
                    ebv_sb = constp.tile([128, VS], F32, tag="ebv")
                    nc.sync.dma_start(
                        out=ebv_sb,
                        in_=bass.AP(tensor=ebv_d, offset=0,
                                    ap=[[0, 128], [1, VS]]))
                    bt_sb = constp.tile([128, T1], F32, tag="bt")
                    nc.sync.dma_start(out=bt_sb, in_=bt_d[:, :])
                Ssl_prev = None
                e_last = None

                for t in range(T1):
                    if t + 1 < T1:
                        fetch_wt(t + 1)
                    xk = xks.pop(t, None)
                    wt_t = wts.pop(t)

                    A = psA.tile([128, 2048], F32, tag="A")
                    lstm_mms(A, hx_prev, xk, hT_prev)

                    # gates in [f|i|g|o] layout, tanh'd in two halves so the
                    # v-stt (needs f) overlaps the second tanh; exps of
                    # h_{t-1} are threaded into the ACT queue so it stays
                    # busy through the DVE/PE recurrence-latency windows
                    th = gp.tile([128, 2048], F32, tag="th")
                    i_tanh1 = nc.scalar.activation(th[:, 0:1024],
                                                   A[:, 0:1024],
                                                   AF.Tanh,
                                                   scale=GSCALE * 0.5)
                    if e_last is not None:
                        # resync: all of step t-1's exps drain before this
                        # step's gate tanh, so a one-off slip (e.g. a late
                        # wv8 chunk) cannot permanently shift exps into the
                        # next period where they'd preempt the gate chain
                        tile_rust.add_dep_helper(
                            i_tanh1.ins, e_last.ins,
                            reason="gate tanh after prev exps")
                    i_tanh2 = nc.scalar.activation(th[:, 1024:2048],
                                                   A[:, 1024:2048],
                                                   AF.Tanh,
                                                   scale=GSCALE * 0.5)
                    v = gp.tile([128, H], F32, tag="v")
                    nc.vector.scalar_tensor_tensor(
                        v, th[:, 0:512], 1.0, c_t, op0=ADD, op1=MUL)
                    if t >= 1:
                        e = emit_vocab(hx_prev, 0, Ssl_prev, sbuf_out=True)
                        if e is not None:
                            tile_rust.add_dep_helper(
                                e.ins, i_tanh2.ins,
                                reason="exp0 after gate tanh2")
                    u = gp.tile([128, H], F32, tag="u")
                    nc.vector.scalar_tensor_tensor(
                        u, th[:, 512:1024], 1.0, th[:, 1024:1536],
                        op0=ADD, op1=MUL)
                    nc.vector.scalar_tensor_tensor(
                        c_t, v, 0.5, u, op0=MUL, op1=ADD)
                    tc_ = gp.tile([128, H], F32, tag="tc")
                    i_tanhC = nc.scalar.activation(tc_, c_t, AF.Tanh,
                                                   scale=0.5)
                    if t >= 1:
                        # explicit edges stop the greedy scheduler from
                        # hoisting these exps ahead of tanhC on the in-order
                        # ACT queue (which would delay the recurrence chain)
                        for vs in (1, 2, 3):
                            e = emit_vocab(hx_prev, vs, Ssl_prev)
                            if e is not None:
                                tile_rust.add_dep_helper(
                                    e.ins, i_tanhC.ins,
                                    reason="exp after tanhC")
                                e_last = e
                        nc.vector.tensor_reduce(
                            out=S_acc[:, t - 1:t], in_=Ssl_prev,
                            axis=mybir.AxisListType.X, op=mybir.AluOpType.add)
                    h_new = hp.tile([128, H], F32, tag="h")
                    nc.vector.scalar_tensor_tensor(
                        h_new, th[:, 1536:2048], 1.0, tc_, op0=ADD, op1=MUL)

                    hx_new = hp.tile([128, 6, 128], FP8, tag="hx8")
                    fetch_x(t + 1, hx_new)
                    hT_new = (None if lstm_fp8 else
                              hp.tile([128, H], F32R, tag="hT"))
                    for b in range(4):
                        nc.tensor.transpose(
                            A[:, 1536 + b * 128:1536 + (b + 1) * 128],
                            h_new[:, b * 128:(b + 1) * 128], ident)
                    i_copy = nc.vector.tensor_copy(hx_new[:, 2:6, :],
                                                   A[:, 1536:2048])
                    if hT_new is not None:
                        nc.vector.tensor_copy(hT_new, A[:, 1536:2048])

                    # target logit: tgt[n] = h_new[n, :] . wt_t[n, :] (+ bt)
                    # dep edges keep these off the h->hx8 recurrence chain
                    prod = scrp.tile([128, H], F32, tag="prod")
                    i_prod = nc.vector.tensor_mul(prod, h_new, wt_t)
                    tile_rust.add_dep_helper(i_prod.ins, i_copy.ins,
                                             reason="tgt after hx copy")
                    if zero_bv:
                        nc.vector.tensor_reduce(
                            out=tgt_acc[:, t:t + 1], in_=prod,
                            axis=mybir.AxisListType.X, op=mybir.AluOpType.add)
                    else:
                        tred = scrp.tile([128, 1], F32, tag="tred")
                        nc.vector.tensor_reduce(
                            out=tred, in_=prod,
                            axis=mybir.AxisListType.X, op=mybir.AluOpType.add)
                        nc.vector.tensor_add(
                            tgt_acc[:, t:t + 1], tred, bt_sb[:, t:t + 1])

                    Ssl_prev = scrp.tile([128, NSL], F32, tag="Ssl")
                    hx_prev, hT_prev = hx_new, hT_new

                for vs in range(NSL):
                    emit_vocab(hx_prev, vs, Ssl_prev)
                nc.vector.tensor_reduce(
                    out=S_acc[:, T1 - 1:T1], in_=Ssl_prev,
                    axis=mybir.AxisListType.X, op=mybir.AluOpType.add)

            nc.sync.dma_start(out=s_d[:, :], in_=S_acc)
            nc.sync.dma_start(out=tgt_d[:, :], in_=tgt_acc)

    nc.finalize()
    return nc


def _prep_inputs(features, captions, W_proj, b_proj, W_embed, Wx, Wh, b,
                 W_vocab, b_vocab, lstm_fp8=LSTM_FP8):
    features = np.asarray(features, dtype=np.float32)
    captions = np.asarray(captions)
    W_proj = np.asarray(W_proj, dtype=np.float32)
    b_proj = np.asarray(b_proj, dtype=np.float32)
    W_embed = np.asarray(W_embed, dtype=np.float32)
    Wx = np.asarray(Wx, dtype=np.float32)
    Wh = np.asarray(Wh, dtype=np.float32)
    b = np.asarray(b, dtype=np.float32)
    W_vocab = np.asarray(W_vocab, dtype=np.float32)
    b_vocab = np.asarray(b_vocab, dtype=np.float32)

    captions_in = captions[:, :-1].astype(np.int64)
    captions_out = captions[:, 1:].astype(np.int64)

    zero_b = bool(np.all(b == 0))
    zero_bp = bool(np.all(b_proj == 0))
    zero_bv = bool(np.all(b_vocab == 0))

    x_emb = W_embed[captions_in]                            # [128, 32, 256]
    # device h/c state carries a 2x factor (all-tanh gates): h0 doubled,
    # Wh halved, and consumers of h descale by 0.5. h0 itself is computed
    # here on host and shipped as the packed (x_0 | 2*h0^T) fp8 tile.
    h0 = (features @ W_proj + b_proj) * 2.0                 # [128, 512]
    h0T = np.ascontiguousarray(h0.T)                        # [512, 128]
    # gathered target columns: wt[t, n, :] = 0.5 * W_vocab[:, captions_out]
    wt = np.ascontiguousarray(
        W_vocab.T[captions_out].transpose(1, 0, 2)) * 0.5   # [32, 128, 512]
    ident = np.eye(128, dtype=np.float32)

    # gate cols permuted to [f|i|g|o]; g block doubled so tanh at scale s/2
    # gives tanh(z/2) on f,i,o and tanh(z_g) on g.
    perm = np.concatenate([
        np.arange(H, 2 * H), np.arange(0, H),
        np.arange(3 * H, 4 * H), np.arange(2 * H, 3 * H)])
    gmul = np.ones(4 * H, dtype=np.float32)
    gmul[2 * H:3 * H] = 2.0

    # packed hx0: rows 0:2 = x_0 (fp8-scaled), rows 2:6 = (2*h0)^T
    hx0 = np.zeros((128, 6, 128), dtype=np.float32)
    hx0[:, 2:6, :] = h0T.reshape(4, 128, 128).transpose(1, 0, 2)
    common = {"wt": wt, "ident": ident}
    if lstm_fp8:
        Wb = np.concatenate(
            [Wx * WX_SCALE, Wh * (WV_SCALE * 0.5)], axis=0)[:, perm] * gmul
        common["wb8"] = np.ascontiguousarray(
            Wb.reshape(6, 128, 4 * H).transpose(1, 0, 2)
        ).astype(ml_dtypes.float8_e4m3)
        xt8 = np.ascontiguousarray(
            (x_emb * X_SCALE).transpose(1, 2, 0)
            .reshape(T1, 2, 128, 128).transpose(0, 2, 1, 3)
        ).astype(ml_dtypes.float8_e4m3)
        common["xt8"] = xt8
        hx0[:, 0:2, :] = xt8[0].astype(np.float32)
    else:
        Wb = np.concatenate([Wx, Wh * 0.5], axis=0)[:, perm] * gmul
        common["wb"] = np.ascontiguousarray(Wb.reshape(6, 128, 4 * H))
        common["xt"] = np.ascontiguousarray(
            x_emb.transpose(1, 2, 0).reshape(T1, 2, 128, 128))
        common["ht0"] = np.ascontiguousarray(
            h0T.reshape(4, 128, 128).transpose(1, 0, 2).reshape(128, H))
    common["hx0"] = hx0.astype(ml_dtypes.float8_e4m3)
    if not zero_b:
        common["ones"] = np.ones((1, 128), dtype=np.float32)
        bscale = WV_SCALE if lstm_fp8 else 1.0
        common["bvec"] = (b[perm] * bscale * gmul).reshape(1, 4 * H)
    if not zero_bv:
        common["bt"] = np.ascontiguousarray(
            b_vocab[captions_out].astype(np.float32))       # [128, 32]

    in_maps = []
    for c in range(NCORES):
        m = dict(common)
        wv_shard = (W_vocab[:, c * VS:(c + 1) * VS] * WV_SCALE)
        m["wv8"] = np.ascontiguousarray(
            wv_shard.reshape(4, 128, VS).transpose(1, 0, 2)
        ).astype(ml_dtypes.float8_e4m3)
        if not zero_bv:
            m["ebv"] = np.exp(
                b_vocab[c * VS:(c + 1) * VS]).reshape(1, VS).astype(np.float32)
        in_maps.append(m)
    return in_maps, captions_out, (zero_b, zero_bp, zero_bv)


def kernel(features, captions, W_proj, b_proj, W_embed, Wx, Wh, b,
           W_vocab, b_vocab):
    in_maps, captions_out, key = _prep_inputs(
        features, captions, W_proj, b_proj, W_embed, Wx, Wh, b,
        W_vocab, b_vocab)
    if key not in _CACHE:
        _CACHE[key] = _build(*key)
    nc = _CACHE[key]

    res = run_bass_kernel_spmd(nc, in_maps, core_ids=list(range(NCORES)))
    global last_results
    last_results = res

    S_total = np.zeros((128, T1), dtype=np.float64)
    for c in range(NCORES):
        S_total += res.results[c]["S_out"].astype(np.float64)
    tgt = res.results[0]["tgt_out"].astype(np.float64)      # [128, 32]
    lse = np.log(S_total)
    mask = (captions_out != NULL)
    loss = (np.where(mask, lse - tgt, 0.0)).sum() / N
    return np.float32(loss)


# revision 67
# speedup vs baseline: 27.3225x; 1.2045x over previous
"""CaptioningRNN forward loss on 8 Trainium2 NeuronCores.

Strategy:
  - The LSTM recurrence is replicated on all 8 cores; the large output
    projection h @ W_vocab (128 x 32 x 512 x 32000) is sharded over the
    vocab axis: each core holds 4000 columns as fp8 * 64 (DoubleRow
    matmuls) and accumulates sum_v exp(logit_v) on-chip via the ACT
    engine's accumulate output, so logits never hit HBM. The target
    logit is computed exactly per row as a DVE dot product with
    host-gathered target columns.
  - All gate nonlinearities are tanh (sigmoid(x) = (tanh(x/2)+1)/2) so
    the ACT engine stays on one activation-table set (exp+tanh): no
    LoadActFuncSet thrash between the gates and the vocab exp. The
    state is carried scaled (c_t = 2c, h = 2h) which makes the gate
    combination the same DVE op count as the sigmoid form; consumers
    descale (W_proj/b_proj doubled, Wh halved, wt halved, exp scale
    halved).
  - One [128, 2048] PSUM tile holds all gate pre-activations in the
    natural [i|f|o|g] layout; the g-block weight columns are doubled on
    host so a single tanh over all 2048 columns at scale s/2 yields
    [tanh(z_ifo/2) | tanh(z_g)] in one ACT instruction.
  - exp work for h_{t-1} overlaps step t's recurrence; the ACT engine
    is the bottleneck and stays ~100% busy.
  - Host combines: loss = sum(mask * (log(sum_cores S) - tgt)) / N.
"""

import numpy as np
import ml_dtypes

import concourse.bass as bass
import concourse.tile as tile
from concourse import mybir, bacc, tile_rust
from concourse.bass_utils import run_bass_kernel_spmd

F32 = mybir.dt.float32
F32R = mybir.dt.float32r
BF16 = mybir.dt.bfloat16
FP8 = mybir.dt.float8e4

# Problem shape (hardcoded per task spec)
N = 128          # batch
T1 = 32          # caption steps (T-1)
D_FEAT = 1280
W_DIM = 256
H = 512
V = 32000
NCORES = 8
VS = V // NCORES          # 4000 vocab cols per core
NSL = 4                   # vocab slices per core (1000 cols each)
SL = VS // NSL            # 1000 cols per slice (2 x 500 halves)
WV_SCALE = 64.0           # W_vocab fp8 scale (descaled in ACT exp)
X_SCALE = 16.0            # x_t fp8 scale   (LSTM fp8 path)
WX_SCALE = 4.0            # Wx fp8 scale    (X_SCALE*WX_SCALE == WV_SCALE)
LSTM_FP8 = True
NULL = 0

_CACHE = {}


def _build(zero_b, zero_bp, zero_bv, repeats=1, lstm_fp8=LSTM_FP8):
    nc = bacc.Bacc("TRN2", target_bir_lowering=False, debug=False)

    if lstm_fp8:
        xt_d = nc.dram_tensor("xt8", [T1, 128, 2, 128], FP8,
                              kind="ExternalInput")
        wb_d = nc.dram_tensor("wb8", [128, 6, 4 * H], FP8,
                              kind="ExternalInput")
    else:
        xt_d = nc.dram_tensor("xt", [T1, 2, 128, 128], F32R,
                              kind="ExternalInput")
        wb_d = nc.dram_tensor("wb", [6, 128, 4 * H], F32R,
                              kind="ExternalInput")
    hx0_d = nc.dram_tensor("hx0", [128, 6, 128], FP8, kind="ExternalInput")
    if not lstm_fp8:
        ht0_d = nc.dram_tensor("ht0", [128, H], F32R, kind="ExternalInput")
    wv_d = nc.dram_tensor("wv8", [128, 4, VS], FP8, kind="ExternalInput")
    wt_d = nc.dram_tensor("wt", [T1, 128, H], BF16, kind="ExternalInput")
    id_d = nc.dram_tensor("ident", [128, 128], F32, kind="ExternalInput")
    if not zero_b:
        ones_d = nc.dram_tensor("ones", [1, 128], F32R, kind="ExternalInput")
        bvec_d = nc.dram_tensor("bvec", [1, 4 * H], F32R, kind="ExternalInput")
    if not zero_bv:
        ebv_d = nc.dram_tensor("ebv", [1, VS], F32, kind="ExternalInput")
        bt_d = nc.dram_tensor("bt", [128, T1], F32, kind="ExternalInput")
    s_d = nc.dram_tensor("S_out", [128, T1], F32, kind="ExternalOutput")
    tgt_d = nc.dram_tensor("tgt_out", [128, T1], F32, kind="ExternalOutput")

    GSCALE = (1.0 / WV_SCALE) if lstm_fp8 else 1.0
    AF = mybir.ActivationFunctionType
    DR = mybir.MatmulPerfMode.DoubleRow
    MUL = mybir.AluOpType.mult
    ADD = mybir.AluOpType.add
    with tile.TileContext(nc) as tc:
        with tc.tile_pool(name="const", bufs=1) as constp, \
             tc.tile_pool(name="wbp", bufs=1) as wbp, \
             tc.tile_pool(name="xk", bufs=3) as xkp, \
             tc.tile_pool(name="wtp", bufs=3) as wtp, \
             tc.tile_pool(name="hpool", bufs=3) as hp, \
             tc.tile_pool(name="gates", bufs=2) as gp, \
             tc.tile_pool(name="scr", bufs=4) as scrp, \
             tc.tile_pool(name="psA", bufs=1, space="PSUM") as psA, \
             tc.tile_pool(name="psV", bufs=2, space="PSUM") as psV:

            # --- resident weights; order = DMA serialization order --------
            # (h0 is computed on HOST and shipped as the tiny packed hx0
            # tile first, so the kernel's critical prefix is just wb8)
            hx_first = hp.tile([128, 6, 128], FP8, tag="hx8")
            nc.sync.dma_start(out=hx_first, in_=hx0_d[:, :, :])
            if lstm_fp8:
                # column-group chunks: with hh-outer matmuls, gate chunk 0
                # is fully loaded after 1/4 of the transfer
                wb8 = constp.tile([128, 6, 4 * H], FP8, tag="wb8")
                for hh in range(4):
                    nc.sync.dma_start(
                        out=wb8[:, :, hh * H:(hh + 1) * H],
                        in_=wb_d[:, :, hh * H:(hh + 1) * H])
            else:
                wb_sb = []
                for k in range(6):
                    wbt = wbp.tile([128, 4 * H], F32R, tag=f"wb{k}")
                    nc.sync.dma_start(out=wbt, in_=wb_d[k])
                    wb_sb.append(wbt)
            ident = constp.tile([128, 128], F32, tag="ident")
            nc.sync.dma_start(out=ident, in_=id_d[:, :])
            if not zero_b:
                ones_sb = constp.tile([1, 128], F32R, tag="ones")
                nc.sync.dma_start(out=ones_sb, in_=ones_d[:, :])
                bvec_sb = constp.tile([1, 4 * H], F32R, tag="bvec")
                nc.sync.dma_start(out=bvec_sb, in_=bvec_d[:, :])

            # persistent state
            c_t = constp.tile([128, H], F32, tag="c")
            S_acc = constp.tile([128, T1], F32, tag="Sacc")
            tgt_acc = constp.tile([128, T1], F32, tag="tgtacc")

            for _rep in range(repeats):
                nc.vector.memset(c_t, 0.0)

                def emit_vocab(hx8, vs, Ssl, sbuf_out=False):
                    # one 1000-col vocab slice: fused (2-pair-row) DR
                    # matmuls, then one exp with accumulate. h carries 2x ->
                    # descale by 2*WV_SCALE. In-place PSUM output is cheaper
                    # than an SBUF ex tile; sbuf_out picks the slower
                    # variant where the ACT queue would idle anyway.
                    pV = psV.tile([128, 2, 512], F32, tag="pV")
                    for hh in range(2):
                        for k in range(2):
                            nc.tensor.matmul(
                                pV[:, hh, 0:500],
                                hx8[:, 2 + 2 * k:4 + 2 * k, :],
                                wv8[:, 2 * k:2 * k + 2,
                                    vs * SL + hh * 500:
                                    vs * SL + (hh + 1) * 500],
                                start=(k == 0), stop=(k == 1),
                                perf_mode=DR)
                    if zero_bv:
                        if sbuf_out:
                            ex = scrp.tile([128, 2, 500], F32, tag="ex")
                            return nc.scalar.activation(
                                ex, pV[:, :, 0:500], AF.Exp,
                                scale=1.0 / (2.0 * WV_SCALE),
                                accum_out=Ssl[:, vs:vs + 1])
                        return nc.scalar.activation(
                            pV[:, :, 0:500], pV[:, :, 0:500], AF.Exp,
                            scale=1.0 / (2.0 * WV_SCALE),
                            accum_out=Ssl[:, vs:vs + 1])
                    ex = scrp.tile([128, 2, 500], F32, tag="ex")
                    nc.scalar.activation(
                        ex, pV[:, :, 0:500], AF.Exp,
                        scale=1.0 / (2.0 * WV_SCALE))
                    exw = scrp.tile([128, 2, 500], F32, tag="exw")
                    ebv_v = ebv_sb.rearrange("p (s hh m) -> p s hh m",
                                             s=NSL, hh=2)
                    nc.vector.tensor_mul(exw, ex, ebv_v[:, vs])
                    nc.vector.tensor_reduce(
                        out=Ssl[:, vs:vs + 1], in_=exw,
                        axis=mybir.AxisListType.XY, op=mybir.AluOpType.add)
                    return None

                def lstm_mms(A, hx8, xk, hT_lhs):
                    # A [128, 2048] = [i|f|o|g] pre-activations (g cols 2x).
                    # fp8: x_t and h^T live packed in one [128, 6, 128] tile
                    # so each 512-col chunk is ONE 6-row (3-pair) DR matmul.
                    # hh-outer order: gate chunk 0 completes after only 3
                    # matmuls, so the gate tanh starts ~1 us sooner on the
                    # recurrence chain
                    nbias = 0 if zero_b else 1
                    if lstm_fp8:
                        for hh in range(4):
                            for k in range(3):
                                nc.tensor.matmul(
                                    A[:, hh * H:(hh + 1) * H],
                                    hx8[:, 2 * k:2 * k + 2, :],
                                    wb8[:, 2 * k:2 * k + 2,
                                        hh * H:(hh + 1) * H],
                                    start=(k == 0),
                                    stop=(k == 2 and nbias == 0),
                                    perf_mode=DR)
                            if not zero_b:
                                nc.tensor.matmul(
                                    A[:, hh * H:(hh + 1) * H], ones_sb,
                                    bvec_sb[:, hh * H:(hh + 1) * H],
                                    start=False, stop=True)
                    else:
                        lhs = [xk[0], xk[1],
                               hT_lhs[:, 0:128], hT_lhs[:, 128:256],
                               hT_lhs[:, 256:384], hT_lhs[:, 384:512]]
                        for hh in range(4):
                            for k in range(6):
                                nc.tensor.matmul(
                                    A[:, hh * H:(hh + 1) * H], lhs[k],
                                    wb_sb[k][:, hh * H:(hh + 1) * H],
                                    start=(k == 0),
                                    stop=(k == 5 and nbias == 0))
                            if not zero_b:
                                nc.tensor.matmul(
                                    A[:, hh * H:(hh + 1) * H], ones_sb,
                                    bvec_sb[:, hh * H:(hh + 1) * H],
                                    start=False, stop=True)

                # stream-load helpers: x_{t} goes straight into the packed
                # [128, 6, 128] hx tile (rows 0:2) whose rows 2:6 later get
                # h_{t-1}^T; wt is the gathered target-column tile.
                xks, wts = {}, {}

                def fetch_x(t, hx):
                    if lstm_fp8:
                        if t < T1:
                            nc.sync.dma_start(out=hx[:, 0:2, :], in_=xt_d[t])
                    else:
                        if t < T1:
                            xk0 = xkp.tile([128, 128], F32R, tag="xk0")
                            nc.sync.dma_start(out=xk0, in_=xt_d[t, 0])
                            xk1 = xkp.tile([128, 128], F32R, tag="xk1")
                            nc.sync.dma_start(out=xk1, in_=xt_d[t, 1])
                            xks[t] = (xk0, xk1)

                def fetch_wt(t):
                    wt_t = wtp.tile([128, H], BF16, tag="wt")
                    nc.sync.dma_start(out=wt_t, in_=wt_d[t])
                    wts[t] = wt_t

                # --- h0 (host-computed, carried as 2*h0): packed hx0 ------
                if _rep == 0:
                    hx_prev = hx_first
                else:
                    hx_prev = hp.tile([128, 6, 128], FP8, tag="hx8")
                    nc.sync.dma_start(out=hx_prev, in_=hx0_d[:, :, :])
                hT_prev = None
                if not lstm_fp8:
                    hT_prev = hp.tile([128, H], F32R, tag="hT")
                    nc.sync.dma_start(out=hT_prev, in_=ht0_d[:, :])
                    fetch_x(0, None)
                fetch_wt(0)
                if _rep == 0:
                    wv8 = constp.tile([128, 4, VS], FP8, tag="wv8")
                    for vs in range(NSL):
                        nc.sync.dma_start(
                            out=wv8[:, :, vs * SL:(vs + 1) * SL],
                            in_=wv_d[:, :, vs * SL:(vs + 1) * SL])
                if not zero_bv and _rep == 0:
                    ebv_sb = constp.tile([128, VS], F32, tag="ebv")
                    nc.sync.dma_start(
                        out=ebv_sb,
                        in_=bass.AP(tensor=ebv_d, offset=0,
                                    ap=[[0, 128], [1, VS]]))
                    bt_sb = constp.tile([128, T1], F32, tag="bt")
                    nc.sync.dma_start(out=bt_sb, in_=bt_d[:, :])
                Ssl_prev = None
                e_last = None

                for t in range(T1):
                    if t + 1 < T1:
                        fetch_wt(t + 1)
                    xk = xks.pop(t, None)
                    wt_t = wts.pop(t)

                    A = psA.tile([128, 2048], F32, tag="A")
                    lstm_mms(A, hx_prev, xk, hT_prev)

                    # gates in [f|i|g|o] layout, tanh'd in two halves so the
                    # v-stt (needs f) overlaps the second tanh; exps of
                    # h_{t-1} are threaded into the ACT queue so it stays
                    # busy through the DVE/PE recurrence-latency windows
                    th = gp.tile([128, 2048], F32, tag="th")
                    i_tanh1 = nc.scalar.activation(th[:, 0:1024],
                                                   A[:, 0:1024],
                                                   AF.Tanh,
                                                   scale=GSCALE * 0.5)
                    if e_last is not None:
                        # resync: all of step t-1's exps drain before this
                        # step's gate tanh, so a one-off slip (e.g. a late
                        # wv8 chunk) cannot permanently shift exps into the
                        # next period where they'd preempt the gate chain
                        tile_rust.add_dep_helper(
                            i_tanh1.ins, e_last.ins,
                            reason="gate tanh after prev exps")
                    i_tanh2 = nc.scalar.activation(th[:, 1024:2048],
                                                   A[:, 1024:2048],
                                                   AF.Tanh,
                                                   scale=GSCALE * 0.5)
                    v = gp.tile([128, H], F32, tag="v")
                    nc.vector.scalar_tensor_tensor(
                        v, th[:, 0:512], 1.0, c_t, op0=ADD, op1=MUL)
                    if t >= 1:
                        e = emit_vocab(hx_prev, 0, Ssl_prev, sbuf_out=True)
                        if e is not None:
                            tile_rust.add_dep_helper(
                                e.ins, i_tanh2.ins,
                                reason="exp0 after gate tanh2")
                    u = gp.tile([128, H], F32, tag="u")
                    nc.vector.scalar_tensor_tensor(
                        u, th[:, 512:1024], 1.0, th[:, 1024:1536],
                        op0=ADD, op1=MUL)
                    nc.vector.scalar_tensor_tensor(
                        c_t, v, 0.5, u, op0=MUL, op1=ADD)
                    tc_ = gp.tile([128, H], F32, tag="tc")
                    i_tanhC = nc.scalar.activation(tc_, c_t, AF.Tanh,
                                                   scale=0.5)
                    if t >= 1:
                        # explicit edges stop the greedy scheduler from
                        # hoisting these exps ahead of tanhC on the in-order
                        # ACT queue (which would delay the recurrence chain)
                        for vs in (1, 2, 3):
                            e = emit_vocab(hx_prev, vs, Ssl_prev)
                            if e is not None:
                                tile_rust.add_dep_helper(
                                    e.ins, i_tanhC.ins,
                                    reason="exp after tanhC")
                                e_last = e
                        nc.vector.tensor_reduce(
                            out=S_acc[:, t - 1:t], in_=Ssl_prev,
                            axis=mybir.AxisListType.X, op=mybir.AluOpType.add)
                    h_new = hp.tile([128, H], F32, tag="h")
                    nc.vector.scalar_tensor_tensor(
                        h_new, th[:, 1536:2048], 1.0, tc_, op0=ADD, op1=MUL)

                    hx_new = hp.tile([128, 6, 128], FP8, tag="hx8")
                    fetch_x(t + 1, hx_new)
                    hT_new = (None if lstm_fp8 else
                              hp.tile([128, H], F32R, tag="hT"))
                    for b in range(4):
                        nc.tensor.transpose(
                            A[:, 1536 + b * 128:1536 + (b + 1) * 128],
                            h_new[:, b * 128:(b + 1) * 128], ident)
                    i_copy = nc.vector.tensor_copy(hx_new[:, 2:6, :],
                                                   A[:, 1536:2048])
                    if hT_new is not None:
                        nc.vector.tensor_copy(hT_new, A[:, 1536:2048])

                    # target logit: tgt[n] = h_new[n, :] . wt_t[n, :] (+ bt)
                    # dep edges keep these off the h->hx8 recurrence chain
                    prod = scrp.tile([128, H], F32, tag="prod")
                    i_prod = nc.vector.tensor_mul(prod, h_new, wt_t)
                    tile_rust.add_dep_helper(i_prod.ins, i_copy.ins,
                                             reason="tgt after hx copy")
                    if zero_bv:
                        nc.vector.tensor_reduce(
                            out=tgt_acc[:, t:t + 1], in_=prod,
                            axis=mybir.AxisListType.X, op=mybir.AluOpType.add)
                    else:
                        tred = scrp.tile([128, 1], F32, tag="tred")
                        nc.vector.tensor_reduce(
                            out=tred, in_=prod,
                            axis=mybir.AxisListType.X, op=mybir.AluOpType.add)
                        nc.vector.tensor_add(
                            tgt_acc[:, t:t + 1], tred, bt_sb[:, t:t + 1])

                    Ssl_prev = scrp.tile([128, NSL], F32, tag="Ssl")
                    hx_prev, hT_prev = hx_new, hT_new

                for vs in range(NSL):
                    emit_vocab(hx_prev, vs, Ssl_prev)
                nc.vector.tensor_reduce(
                    out=S_acc[:, T1 - 1:T1], in_=Ssl_prev,
                    axis=mybir.AxisListType.X, op=mybir.AluOpType.add)

            nc.sync.dma_start(out=s_d[:, :], in_=S_acc)
            nc.sync.dma_start(out=tgt_d[:, :], in_=tgt_acc)

    nc.finalize()
    return nc


def _prep_inputs(features, captions, W_proj, b_proj, W_embed, Wx, Wh, b,
                 W_vocab, b_vocab, lstm_fp8=LSTM_FP8):
    features = np.asarray(features, dtype=np.float32)
    captions = np.asarray(captions)
    W_proj = np.asarray(W_proj, dtype=np.float32)
    b_proj = np.asarray(b_proj, dtype=np.float32)
    W_embed = np.asarray(W_embed, dtype=np.float32)
    Wx = np.asarray(Wx, dtype=np.float32)
    Wh = np.asarray(Wh, dtype=np.float32)
    b = np.asarray(b, dtype=np.float32)
    W_vocab = np.asarray(W_vocab, dtype=np.float32)
    b_vocab = np.asarray(b_vocab, dtype=np.float32)

    captions_in = captions[:, :-1].astype(np.int64)
    captions_out = captions[:, 1:].astype(np.int64)

    zero_b = bool(np.all(b == 0))
    zero_bp = bool(np.all(b_proj == 0))
    zero_bv = bool(np.all(b_vocab == 0))

    x_emb = W_embed[captions_in]                            # [128, 32, 256]
    # device h/c state carries a 2x factor (all-tanh gates): h0 doubled,
    # Wh halved, and consumers of h descale by 0.5. h0 itself is computed
    # here on host and shipped as the packed (x_0 | 2*h0^T) fp8 tile.
    h0 = (features @ W_proj + b_proj) * 2.0                 # [128, 512]
    h0T = np.ascontiguousarray(h0.T)                        # [512, 128]
    # gathered target columns: wt[t, n, :] = 0.5 * W_vocab[:, captions_out]
    wt = (np.ascontiguousarray(
        W_vocab.T[captions_out].transpose(1, 0, 2)) * 0.5
    ).astype(ml_dtypes.bfloat16)                            # [32, 128, 512]
    ident = np.eye(128, dtype=np.float32)

    # gate cols permuted to [f|i|g|o]; g block doubled so tanh at scale s/2
    # gives tanh(z/2) on f,i,o and tanh(z_g) on g.
    perm = np.concatenate([
        np.arange(H, 2 * H), np.arange(0, H),
        np.arange(3 * H, 4 * H), np.arange(2 * H, 3 * H)])
    gmul = np.ones(4 * H, dtype=np.float32)
    gmul[2 * H:3 * H] = 2.0

    # packed hx0: rows 0:2 = x_0 (fp8-scaled), rows 2:6 = (2*h0)^T
    hx0 = np.zeros((128, 6, 128), dtype=np.float32)
    hx0[:, 2:6, :] = h0T.reshape(4, 128, 128).transpose(1, 0, 2)
    common = {"wt": wt, "ident": ident}
    if lstm_fp8:
        Wb = np.concatenate(
            [Wx * WX_SCALE, Wh * (WV_SCALE * 0.5)], axis=0)[:, perm] * gmul
        common["wb8"] = np.ascontiguousarray(
            Wb.reshape(6, 128, 4 * H).transpose(1, 0, 2)
        ).astype(ml_dtypes.float8_e4m3)
        xt8 = np.ascontiguousarray(
            (x_emb * X_SCALE).transpose(1, 2, 0)
            .reshape(T1, 2, 128, 128).transpose(0, 2, 1, 3)
        ).astype(ml_dtypes.float8_e4m3)
        common["xt8"] = xt8
        hx0[:, 0:2, :] = xt8[0].astype(np.float32)
    else:
        Wb = np.concatenate([Wx, Wh * 0.5], axis=0)[:, perm] * gmul
        common["wb"] = np.ascontiguousarray(Wb.reshape(6, 128, 4 * H))
        common["xt"] = np.ascontiguousarray(
            x_emb.transpose(1, 2, 0).reshape(T1, 2, 128, 128))
        common["ht0"] = np.ascontiguousarray(
            h0T.reshape(4, 128, 128).transpose(1, 0, 2).reshape(128, H))
    common["hx0"] = hx0.astype(ml_dtypes.float8_e4m3)
    if not zero_b:
        common["ones"] = np.ones((1, 128), dtype=np.float32)
        bscale = WV_SCALE if lstm_fp8 else 1.0
        common["bvec"] = (b[perm] * bscale * gmul).reshape(1, 4 * H)
    if not zero_bv:
        common["bt"] = np.ascontiguousarray(
            b_vocab[captions_out].astype(np.float32))       # [128, 32]

    in_maps = []
    for c in range(NCORES):
        m = dict(common)
        wv_shard = (W_vocab[:, c * VS:(c + 1) * VS] * WV_SCALE)
        m["wv8"] = np.ascontiguousarray(
            wv_shard.reshape(4, 128, VS).transpose(1, 0, 2)
        ).astype(ml_dtypes.float8_e4m3)
        if not zero_bv:
            m["ebv"] = np.exp(
                b_vocab[c * VS:(c + 1) * VS]).reshape(1, VS).astype(np.float32)
        in_maps.append(m)
    return in_maps, captions_out, (zero_b, zero_bp, zero_bv)


def kernel(features, captions, W_proj, b_proj, W_embed, Wx, Wh, b,
           W_vocab, b_vocab):
    in_maps, captions_out, key = _prep_inputs(
        features, captions, W_proj, b_proj, W_embed, Wx, Wh, b,
        W_vocab, b_vocab)
    if key not in _CACHE:
        _CACHE[key] = _build(*key)
    nc = _CACHE[key]

    res = run_bass_kernel_spmd(nc, in_maps, core_ids=list(range(NCORES)))
    global last_results
    last_results = res

    S_total = np.zeros((128, T1), dtype=np.float64)
    for c in range(NCORES):
        S_total += res.results[c]["S_out"].astype(np.float64)
    tgt = res.results[0]["tgt_out"].astype(np.float64)      # [128, 32]
    lse = np.log(S_total)
    mask = (captions_out != NULL)
    loss = (np.where(mask, lse - tgt, 0.0)).sum() / N
    return np.float32(loss)
